# revision 1
# baseline (speedup 1.0000x reference)
"""Trainium2 Bass kernel for nn_DeepConvGraphEncoderPre.

Model: 4x GCN (dense normalized adjacency) -> mean-pool over nodes ->
single-step BiLSTM -> fc -> temporal attention over T -> linear head.

Sharding: data-parallel over batch B=8 across 8 NeuronCores (1 batch row
per core).  edge_index and all weights replicated.  The normalized dense
adjacency A^T [256,256] is built ON DEVICE from edge_index via one-hot
matmuls (exact, handles duplicate edges), then every GCN layer is two
dense matmuls (aggregate-first):  x <- relu((A x) W + b).

All heavy matmuls run in float32r (full PE speed at free-dim >= 256).
"""

import numpy as np

B, T, N, F, E = 8, 32, 256, 64, 4096
H, EMB, OUT = 256, 256, 512
NCORES = 8
NPAIR = T // 2  # graph pairs per core

_CACHE = {}
RUN_KWARGS = {}   # test harness may set {"trace": True, ...}
LAST_RESULT = None


def _build(flags):
    import concourse.mybir as mybir
    import concourse.tile as tile
    from concourse import bacc
    from concourse.masks import make_identity

    dt = mybir.dt
    f32, f32r, bf16, i32 = dt.float32, dt.float32r, dt.bfloat16, dt.int32
    AF = mybir.ActivationFunctionType
    ALU = mybir.AluOpType

    gcn_bias, lstm_bias, fc_bias, out_bias = (
        flags["gcn_bias"], flags["lstm_bias"], flags["fc_bias"], flags["out_bias"])
    stage = flags.get("stage", "full")
    nlayers = flags.get("nlayers", 4)
    npair_run = flags.get("npair", NPAIR)

    nc = bacc.Bacc("TRN2", target_bir_lowering=False, debug=False,
                   num_devices=NCORES)

    def r(ap):
        return ap.bitcast(f32r)

    def rf(ap):
        return ap.bitcast(f32)

    # ---------------- DRAM I/O ----------------
    data_d = nc.dram_tensor("data_local", [T, N, F], f32r, kind="ExternalInput")
    edge_d = nc.dram_tensor("edge_index", [2, E], i32, kind="ExternalInput")
    W_d = [nc.dram_tensor(f"W{i+1}", [c_in, c_out], f32r, kind="ExternalInput")
           for i, (c_in, c_out) in enumerate([(64, 64), (64, 128), (128, 256), (256, 256)])]
    Wih_d = {d: nc.dram_tensor(f"W_ih_{d}", [4 * H, H], f32, kind="ExternalInput")
             for d in ("f", "b")}
    fcW_d = nc.dram_tensor("fc_W", [2 * H, EMB], f32r, kind="ExternalInput")
    attnW_d = nc.dram_tensor("attn_W", [EMB, 1], f32r, kind="ExternalInput")
    outW_d = nc.dram_tensor("out_W", [EMB, OUT], f32r, kind="ExternalInput")
    if gcn_bias:
        b_d = [nc.dram_tensor(f"b{i+1}", [c], f32, kind="ExternalInput")
               for i, c in enumerate([64, 128, 256, 256])]
    if lstm_bias:
        bih_d = {d: nc.dram_tensor(f"b_ih_{d}", [4 * H], f32, kind="ExternalInput")
                 for d in ("f", "b")}
        bhh_d = {d: nc.dram_tensor(f"b_hh_{d}", [4 * H], f32, kind="ExternalInput")
                 for d in ("f", "b")}
    if fc_bias:
        fcb_d = nc.dram_tensor("fc_b", [EMB], f32r, kind="ExternalInput")
    if out_bias:
        outb_d = nc.dram_tensor("out_b", [OUT], f32r, kind="ExternalInput")
    out_d = nc.dram_tensor("out", [1, OUT], f32, kind="ExternalOutput")

    taps = {}
    if flags.get("debug_taps"):
        taps["AT"] = nc.dram_tensor("tap_AT", [128, 512], f32r, kind="ExternalOutput")
        taps["x1"] = nc.dram_tensor("tap_x1", [128, 256], f32r, kind="ExternalOutput")
        taps["pooledT"] = nc.dram_tensor("tap_pooledT", [128, 64], f32r, kind="ExternalOutput")
        taps["h"] = nc.dram_tensor("tap_h", [T, 2 * H], f32, kind="ExternalOutput")
        taps["emb"] = nc.dram_tensor("tap_emb", [T, EMB], f32r, kind="ExternalOutput")
        taps["w"] = nc.dram_tensor("tap_w", [1, T], f32r, kind="ExternalOutput")

    with tile.TileContext(nc) as tc:
        # ================= persistent constants =================
        with tc.tile_pool(name="const", bufs=1) as cp:
            AT_sb = cp.tile([128, 512], f32r)          # col k*256+d ; A^T[s,d], s=k*128+p
            W1_sb = cp.tile([64, 64], f32r)
            W2_sb = cp.tile([64, 128], f32r)
            W3_sb = cp.tile([128, 256], f32r)
            W4_sb = cp.tile([128, 512], f32r)          # col k*256+co
            WihT_sb = [cp.tile([128, 2048], f32r, name=f"WihT{k}") for k in (0, 1)]
            fcW_sb = cp.tile([128, 1024], f32r)        # col k*256+m
            outW_sb = cp.tile([128, 1024], f32r)       # col m*512+o
            attnW_sb = cp.tile([128, 2], f32r)         # col m
            ones_col = cp.tile([128, 1], f32r)
            ones_11 = cp.tile([1, 1], f32r)
            ident = cp.tile([128, 128], f32)
            pooledT_sb = cp.tile([128, 64], f32r)      # col mo*32 + t
            if gcn_bias:
                bb_sb = [cp.tile([128, 256], f32, name=f"bb{i}") for i in range(3)]
                b4col_sb = cp.tile([128, 2], f32)
            if lstm_bias:
                lbias_sb = cp.tile([1, 2048], f32r)    # col gate*512+dir*256+h
            if fc_bias:
                fcb_row = cp.tile([1, EMB], f32r)
                fcb_col = cp.tile([128, 2], f32)
            if out_bias:
                outb_row = cp.tile([1, OUT], f32r)

            ones_f = cp.tile([128, 1], f32)
            nc.gpsimd.memset(ones_f[:], 1.0)
            nc.vector.tensor_copy(ones_col[:], ones_f[:])
            nc.vector.tensor_copy(ones_11[:], ones_f[0:1, :])
            make_identity(nc, ident[:])

            # ---- weight loads (natural layouts) ----
            nc.sync.dma_start(out=W1_sb[:], in_=W_d[0].ap())
            nc.sync.dma_start(out=W2_sb[:], in_=W_d[1].ap())
            nc.sync.dma_start(out=W3_sb[:], in_=W_d[2].ap())
            nc.sync.dma_start(
                out=W4_sb[:].rearrange("p (k co) -> p k co", k=2),
                in_=W_d[3].ap().rearrange("(k p) co -> p k co", p=128))
            nc.sync.dma_start(
                out=fcW_sb[:].rearrange("p (k m) -> p k m", k=4),
                in_=fcW_d.ap().rearrange("(k p) m -> p k m", p=128))
            nc.sync.dma_start(
                out=outW_sb[:].rearrange("p (m o) -> p m o", m=2),
                in_=outW_d.ap().rearrange("(m p) o -> p m o", p=128))
            nc.sync.dma_start(
                out=attnW_sb[:].unsqueeze(2),
                in_=attnW_d.ap().rearrange("(m p) one -> p m one", p=128))
            if gcn_bias:
                pass  # filled below with broadcasts
            if fc_bias:
                nc.sync.dma_start(out=fcb_row[:], in_=fcb_d.ap().rearrange("m -> 1 m"))
                nc.sync.dma_start(
                    out=fcb_col[:].unsqueeze(2),
                    in_=fcb_d.ap().rearrange("(m p) -> p m", p=128).unsqueeze(2))
            if out_bias:
                nc.sync.dma_start(out=outb_row[:], in_=outb_d.ap().rearrange("o -> 1 o"))

            # ============ stage 0: A^T build + W_ih transpose ============
            with (
                tc.tile_pool(name="ab_sb", bufs=2) as ab,
                tc.tile_pool(name="ab_ps", bufs=1, space="PSUM") as abp,
                tc.tile_pool(name="wt_ps", bufs=2, space="PSUM") as wtp,
                tc.tile_pool(name="oh", bufs=4) as ohp,
            ):
                # iota row 0..255 (int32 -> bf16), broadcast to 128 partitions
                iota_i = ab.tile([1, 256], i32)
                nc.gpsimd.iota(iota_i[:], pattern=[[1, 256]], base=0,
                               channel_multiplier=0)
                iota_b = ab.tile([1, 256], bf16)
                nc.vector.tensor_copy(iota_b[:], iota_i[:])
                iota_bc = ab.tile([128, 256], bf16)
                nc.gpsimd.partition_broadcast(iota_bc[:], iota_b[:])

                # edge index columns [128, 32] per row, cast to bf16
                eg_i = ab.tile([128, 64], i32)   # col j<32: src ; col 32+j: dst
                nc.sync.dma_start(
                    out=eg_i[:, 0:32],
                    in_=edge_d.ap().rearrange("two (p j) -> two p j", p=128)[0])
                nc.sync.dma_start(
                    out=eg_i[:, 32:64],
                    in_=edge_d.ap().rearrange("two (p j) -> two p j", p=128)[1])
                eg_b = ab.tile([128, 64], f32)
                nc.vector.tensor_copy(eg_b[:], eg_i[:])

                # self-loop columns p+128k
                sl_i = ab.tile([128, 2], i32)
                nc.gpsimd.iota(sl_i[:], pattern=[[128, 2]], base=0,
                               channel_multiplier=1)
                sl_b = ab.tile([128, 2], f32)
                nc.vector.tensor_copy(sl_b[:], sl_i[:])

                # accumulate A^T_unnorm = sum_e onehot_src^T(slice) @ onehot_dst
                atun_ps = abp.tile([128, 512], f32)
                for c in range(34):
                    if c < 32:
                        scol = eg_b[:, c:c + 1]
                        dcol = eg_b[:, 32 + c:33 + c]
                    else:
                        scol = dcol = sl_b[:, c - 32:c - 31]
                    oh_s = ohp.tile([128, 256], bf16, tag="ohs")
                    nc.vector.tensor_scalar(oh_s[:], iota_bc[:], scol, None,
                                            op0=ALU.is_equal)
                    if c < 32:
                        oh_d = ohp.tile([128, 256], bf16, tag="ohd")
                        nc.vector.tensor_scalar(oh_d[:], iota_bc[:], dcol, None,
                                                op0=ALU.is_equal)
                    else:
                        oh_d = oh_s
                    for m in (0, 1):
                        nc.tensor.matmul(atun_ps[:, m * 256:(m + 1) * 256],
                                         oh_s[:, m * 128:(m + 1) * 128], oh_d[:],
                                         start=(c == 0 and m == 0),
                                         stop=(c == 33 and m == 1))
                atun_sb = ab.tile([128, 512], f32r)
                nc.scalar.copy(atun_sb[:], atun_ps[:])

                # deg (row + col forms), dinv = 1/sqrt(deg)   (deg >= 1 always)
                deg_ps = abp.tile([1, 256], f32, tag="deg")
                for m in (0, 1):
                    nc.tensor.matmul(deg_ps[:], rf(ones_col[:]),
                                     rf(atun_sb[:, m * 256:(m + 1) * 256]),
                                     start=(m == 0), stop=(m == 1))
                degc_ps = abp.tile([128, 2], f32, tag="degc")
                for dm in (0, 1):
                    for m in (0, 1):
                        nc.tensor.matmul(
                            degc_ps[:, dm:dm + 1],
                            rf(atun_sb[:, m * 256 + dm * 128: m * 256 + (dm + 1) * 128]),
                            rf(ones_col[:]), start=(m == 0), stop=(m == 1))
                dinv_row = ab.tile([1, 256], f32)
                nc.vector.reciprocal(dinv_row[:], deg_ps[:])
                nc.scalar.sqrt(dinv_row[:], dinv_row[:])
                dinv_col = ab.tile([128, 2], f32)
                nc.vector.reciprocal(dinv_col[:], degc_ps[:])
                nc.scalar.sqrt(dinv_col[:], dinv_col[:])
                dinv_bc = ab.tile([128, 256], f32)
                nc.gpsimd.partition_broadcast(dinv_bc[:], dinv_row[:])

                # AT_norm[s,d] = dinv[s] * ATun[s,d] * dinv[d]
                for m in (0, 1):
                    nc.vector.scalar_tensor_tensor(
                        out=AT_sb[:, m * 256:(m + 1) * 256],
                        in0=atun_sb[:, m * 256:(m + 1) * 256],
                        scalar=dinv_col[:, m:m + 1],
                        in1=dinv_bc[:],
                        op0=ALU.mult, op1=ALU.mult)

                if "AT" in taps:
                    nc.sync.dma_start(out=taps["AT"].ap(), in_=AT_sb[:])
                if stage == "abuild":
                    nc.sync.dma_start(out=out_d.ap(), in_=AT_sb[0:1, 0:512].bitcast(f32))

                # ---- W_ih transpose (PE) into WihT_sb, scaled by 1/N ----
                # WihT_sb[ks] col = gate*512 + dir*256 + h  (h = hh*128 + p)
                for di, d in enumerate(("f", "b")):
                    wnat = ab.tile([128, 2048], f32, tag="wnat")
                    nc.sync.dma_start(
                        out=wnat[:].rearrange("p (i c) -> p i c", i=8),
                        in_=Wih_d[d].ap().rearrange("(i p) c -> p i c", p=128))
                    for ks in (0, 1):
                        wt_ps = wtp.tile([128, 1024], f32, tag="wt")
                        for i in range(8):
                            nc.tensor.transpose(
                                wt_ps[:, i * 128:(i + 1) * 128],
                                wnat[:, i * 256 + ks * 128: i * 256 + (ks + 1) * 128],
                                ident[:])
                        # evac with (gate,hh) regrouping + 1/N scale
                        dst = WihT_sb[ks][:].rearrange(
                            "p (g d hh r) -> p g d hh r", g=4, d=2, hh=2)[:, :, di]
                        nc.scalar.mul(
                            dst,
                            wt_ps[:].rearrange("p (g hh r) -> p g hh r", g=4, hh=2),
                            1.0 / N)

                # ---- bias prep ----
                if gcn_bias:
                    # bb_sb[0]: col m*128+g*64+c -> b1[c] repeated 4x
                    b1r = ab.tile([1, 64], f32, tag="b1r")
                    nc.sync.dma_start(out=b1r[:], in_=b_d[0].ap().rearrange("c -> 1 c"))
                    brow = ab.tile([1, 256], f32, tag="brow")
                    nc.vector.tensor_copy(
                        brow[:].rearrange("one (r c) -> one r c", r=4),
                        b1r[:].rearrange("one c -> one 1 c").broadcast_to([1, 4, 64]))
                    nc.gpsimd.partition_broadcast(bb_sb[0][:], brow[:])
                    # bb_sb[1]: col m*256+g*128+c -> b2[c] repeated 2x
                    b2r = ab.tile([1, 128], f32, tag="b2r")
                    nc.sync.dma_start(out=b2r[:], in_=b_d[1].ap().rearrange("c -> 1 c"))
                    brow2 = ab.tile([1, 256], f32, tag="brow")
                    nc.vector.tensor_copy(
                        brow2[:].rearrange("one (r c) -> one r c", r=2),
                        b2r[:].rearrange("one c -> one 1 c").broadcast_to([1, 2, 128]))
                    nc.gpsimd.partition_broadcast(bb_sb[1][:], brow2[:])
                    brow3 = ab.tile([1, 256], f32, tag="brow")
                    nc.sync.dma_start(out=brow3[:],
                                      in_=b_d[2].ap().rearrange("c -> 1 c"))
                    nc.gpsimd.partition_broadcast(bb_sb[2][:], brow3[:])
                    nc.sync.dma_start(
                        out=b4col_sb[:].unsqueeze(2),
                        in_=b_d[3].ap().rearrange("(m p) -> p m", p=128).unsqueeze(2))
                if lstm_bias:
                    for di, d in enumerate(("f", "b")):
                        bi = ab.tile([1, 1024], f32, tag="lbias_i")
                        bh = ab.tile([1, 1024], f32, tag="lbias_h")
                        nc.sync.dma_start(out=bi[:], in_=bih_d[d].ap().rearrange("g -> 1 g"))
                        nc.sync.dma_start(out=bh[:], in_=bhh_d[d].ap().rearrange("g -> 1 g"))
                        nc.vector.tensor_add(
                            lbias_sb[:, di * 256:].rearrange("one (g q) -> one g q", g=4)[:, :, 0:256],
                            bi[:].rearrange("one (g q) -> one g q", g=4),
                            bh[:].rearrange("one (g q) -> one g q", g=4))

            # ================= input prefetch =================
            x0_all = cp.tile([128, NPAIR * 256], f32r)  # col j*256+k*128+g*64+c
            dv = data_d.ap().rearrange("(j two) (k p) c -> two k p j c", two=2, p=128)
            xv = x0_all[:].rearrange("p (j k g c) -> k g p j c", j=NPAIR, k=2, g=2)
            for k in (0, 1):
                for g in (0, 1):
                    nc.sync.dma_start(out=xv[k, g], in_=dv[g, k])

            # ================= main GCN loop (graph pairs) =================
            run_gcn = stage in ("gcn", "lstm", "full")
            if not run_gcn:
                pass
            with (
                tc.tile_pool(name="work", bufs=2) as wk,
                tc.tile_pool(name="psA", bufs=2, space="PSUM") as psA,
                tc.tile_pool(name="psB", bufs=2, space="PSUM") as psB,
                tc.tile_pool(name="psC", bufs=2, space="PSUM") as psC,
            ):
                for j in range(npair_run if run_gcn else 0):
                    xb = j * 256

                    # ---- L1 ----  (per-graph M=64 agg; all stationaries at base partition 0)
                    agg1 = psC.tile([64, 512], f32, tag="C")
                    for g in (0, 1):
                        for k in (0, 1):
                            nc.tensor.matmul(
                                agg1[:, g * 256:(g + 1) * 256],
                                r(x0_all[:, xb + k * 128 + g * 64: xb + k * 128 + (g + 1) * 64]),
                                r(AT_sb[:, k * 256:(k + 1) * 256]),
                                start=(k == 0), stop=(k == 1))
                    agg1_sb = wk.tile([64, 512], f32r, tag="agg1sb")
                    nc.scalar.copy(agg1_sb[:], agg1[:])
                    z1 = psC.tile([128, 256], f32, tag="C")
                    for g in (0, 1):
                        for m in (0, 1):
                            nc.tensor.matmul(
                                z1[:, m * 128 + g * 64: m * 128 + (g + 1) * 64],
                                r(agg1_sb[0:64, g * 256 + m * 128: g * 256 + (m + 1) * 128]),
                                r(W1_sb[:]), start=True, stop=True)
                    x1 = wk.tile([128, 256], f32r, tag="x1")
                    if gcn_bias:
                        nc.vector.tensor_add(x1[:], z1[:], bb_sb[0][:])
                        nc.vector.tensor_relu(x1[:], x1[:])
                    else:
                        nc.vector.tensor_relu(x1[:], z1[:])
                    if "x1" in taps and j == 0:
                        nc.sync.dma_start(out=taps["x1"].ap(), in_=x1[:])

                    if nlayers == 1:
                        if j == 0:
                            nc.sync.dma_start(out=out_d.ap()[0:1, 0:256],
                                              in_=x1[0:1, :].bitcast(f32))
                        continue
                    # ---- L2 ----
                    agg2 = psC.tile([64, 512], f32, tag="C")
                    for g in (0, 1):
                        for k in (0, 1):
                            nc.tensor.matmul(
                                agg2[:, g * 256:(g + 1) * 256],
                                r(x1[:, k * 128 + g * 64: k * 128 + (g + 1) * 64]),
                                r(AT_sb[:, k * 256:(k + 1) * 256]),
                                start=(k == 0), stop=(k == 1))
                    agg2_sb = wk.tile([64, 512], f32r, tag="agg2sb")
                    nc.scalar.copy(agg2_sb[:], agg2[:])
                    z2 = psB.tile([128, 512], f32, tag="B")
                    for g in (0, 1):
                        for m in (0, 1):
                            nc.tensor.matmul(
                                z2[:, m * 256 + g * 128: m * 256 + (g + 1) * 128],
                                r(agg2_sb[0:64, g * 256 + m * 128: g * 256 + (m + 1) * 128]),
                                r(W2_sb[:]), start=True, stop=True)
                    x2 = wk.tile([128, 512], f32r, tag="x2")
                    if gcn_bias:
                        nc.vector.tensor_add(
                            x2[:].rearrange("p (m q) -> p m q", m=2),
                            z2[:].rearrange("p (m q) -> p m q", m=2),
                            bb_sb[1][:].rearrange("p q -> p 1 q").broadcast_to([128, 2, 256]))
                        nc.vector.tensor_relu(x2[:], x2[:])
                    else:
                        nc.vector.tensor_relu(x2[:], z2[:])

                    if nlayers == 2:
                        if j == 0:
                            nc.sync.dma_start(out=out_d.ap()[0:1, 0:512],
                                              in_=x2[0:1, :].bitcast(f32))
                        continue
                    # ---- L3 ----
                    agg3 = psB.tile([128, 512], f32, tag="B")
                    for g in (0, 1):
                        for k in (0, 1):
                            nc.tensor.matmul(
                                agg3[:, g * 256:(g + 1) * 256],
                                r(x2[:, k * 256 + g * 128: k * 256 + (g + 1) * 128]),
                                r(AT_sb[:, k * 256:(k + 1) * 256]),
                                start=(k == 0), stop=(k == 1))
                    agg3_sb = wk.tile([128, 512], f32r, tag="agg3sb")
                    nc.scalar.copy(agg3_sb[:], agg3[:])
                    z3 = psA.tile([128, 1024], f32, tag="A")
                    for g in (0, 1):
                        for m in (0, 1):
                            nc.tensor.matmul(
                                z3[:, g * 512 + m * 256: g * 512 + (m + 1) * 256],
                                r(agg3_sb[:, g * 256 + m * 128: g * 256 + (m + 1) * 128]),
                                r(W3_sb[:]), start=True, stop=True)
                    x3 = wk.tile([128, 1024], f32r, tag="x3")
                    if gcn_bias:
                        nc.vector.tensor_add(
                            x3[:].rearrange("p (gm q) -> p gm q", gm=4),
                            z3[:].rearrange("p (gm q) -> p gm q", gm=4),
                            bb_sb[2][:].rearrange("p q -> p 1 q").broadcast_to([128, 4, 256]))
                        nc.vector.tensor_relu(x3[:], x3[:])
                    else:
                        nc.vector.tensor_relu(x3[:], z3[:])

                    if nlayers == 3:
                        if j == 0:
                            nc.sync.dma_start(out=out_d.ap()[0:1, 0:512],
                                              in_=x3[0:1, 0:512].bitcast(f32))
                        continue
                    # ---- L4 ----
                    agg4 = psA.tile([128, 1024], f32, tag="A")
                    for g in (0, 1):
                        for mc in (0, 1):
                            for k in (0, 1):
                                nc.tensor.matmul(
                                    agg4[:, g * 512 + mc * 256: g * 512 + (mc + 1) * 256],
                                    r(x3[:, g * 512 + k * 256 + mc * 128:
                                          g * 512 + k * 256 + (mc + 1) * 128]),
                                    r(AT_sb[:, k * 256:(k + 1) * 256]),
                                    start=(k == 0), stop=(k == 1))
                    agg4_sb = wk.tile([128, 1024], f32r, tag="agg4sb")
                    nc.scalar.copy(agg4_sb[:], agg4[:])
                    z4 = psA.tile([128, 1024], f32, tag="A")
                    for g in (0, 1):
                        for mo in (0, 1):
                            for k in (0, 1):
                                nc.tensor.matmul(
                                    z4[:, g * 512 + mo * 256: g * 512 + (mo + 1) * 256],
                                    r(W4_sb[:, k * 256 + mo * 128: k * 256 + (mo + 1) * 128]),
                                    r(agg4_sb[:, g * 512 + k * 256: g * 512 + (k + 1) * 256]),
                                    start=(k == 0), stop=(k == 1))
                    x4 = wk.tile([128, 1024], f32, tag="x4")
                    for g in (0, 1):
                        t_idx = 2 * j + g
                        for mo in (0, 1):
                            ctx_lp = nc.allow_low_precision(reason="fp32r accum (32-bit)")
                            ctx_lp.__enter__()
                            nc.scalar.activation(
                                x4[:, g * 512 + mo * 256: g * 512 + (mo + 1) * 256],
                                z4[:, g * 512 + mo * 256: g * 512 + (mo + 1) * 256],
                                AF.Relu,
                                bias=(b4col_sb[:, mo:mo + 1] if gcn_bias else 0.0),
                                accum_out=pooledT_sb[:, mo * 32 + t_idx: mo * 32 + t_idx + 1])
                            ctx_lp.__exit__(None, None, None)

            if "pooledT" in taps:
                nc.sync.dma_start(out=taps["pooledT"].ap(), in_=pooledT_sb[:])
            if stage == "gcn" and nlayers == 4:
                nc.sync.dma_start(out=out_d.ap()[0:1, 0:64],
                                  in_=pooledT_sb[0:1, :].bitcast(f32))

            # ================= LSTM + fc + attention + head =================
            run_tail = stage in ("lstm", "full")
            if run_tail:
                with (
                    tc.tile_pool(name="tail", bufs=1) as tl,
                    tc.tile_pool(name="tailps_g", bufs=1, space="PSUM") as tpg,
                    tc.tile_pool(name="tailps", bufs=2, space="PSUM") as tp,
                ):
                    # gates [32, 2048] col gate*512 + dir*256 + h  (pooled already /N)
                    g_ps = tpg.tile([32, 2048], f32, tag="gates")
                    if lstm_bias:
                        ones_r32 = tl.tile([1, 32], f32r)
                        ones_r32f = tl.tile([1, 32], f32)
                        nc.gpsimd.memset(ones_r32f[:], 1.0)
                        nc.vector.tensor_copy(ones_r32[:], ones_r32f[:])
                    for s in range(4):
                        for k in (0, 1):
                            nc.tensor.matmul(
                                g_ps[:, s * 512:(s + 1) * 512],
                                r(pooledT_sb[:, k * 32:(k + 1) * 32]),
                                r(WihT_sb[k][:, s * 512:(s + 1) * 512]),
                                start=(k == 0), stop=(k == 1 and not lstm_bias))
                        if lstm_bias:
                            nc.tensor.matmul(g_ps[:, s * 512:(s + 1) * 512],
                                             r(ones_r32[:]),
                                             r(lbias_sb[:, s * 512:(s + 1) * 512]),
                                             start=False, stop=True)
                    sig_i = tl.tile([32, 512], f32)
                    tanh_g = tl.tile([32, 512], f32)
                    sig_o = tl.tile([32, 512], f32)
                    nc.scalar.activation(sig_i[:], g_ps[:, 0:512], AF.Sigmoid)
                    nc.scalar.activation(tanh_g[:], g_ps[:, 1024:1536], AF.Tanh)
                    nc.scalar.activation(sig_o[:], g_ps[:, 1536:2048], AF.Sigmoid)
                    c_sb = tl.tile([32, 512], f32)
                    nc.vector.tensor_mul(c_sb[:], sig_i[:], tanh_g[:])
                    tc_sb = tl.tile([32, 512], f32)
                    nc.scalar.activation(tc_sb[:], c_sb[:], AF.Tanh)
                    h_sb = tl.tile([32, 512], f32)
                    nc.vector.tensor_mul(h_sb[:], sig_o[:], tc_sb[:])
                    if "h" in taps:
                        nc.sync.dma_start(out=taps["h"].ap(), in_=h_sb[:])

                    # transpose h -> hT [128, (k,t)]
                    hT_ps = tp.tile([128, 128], f32, tag="small")
                    for k in range(4):
                        nc.tensor.transpose(hT_ps[:, k * 32:(k + 1) * 32],
                                            h_sb[:, k * 128:(k + 1) * 128],
                                            ident[0:32, 0:32])
                    hT_sb = tl.tile([128, 128], f32r)
                    nc.vector.tensor_copy(hT_sb[:], hT_ps[:])

                    # emb (node-major) [32, 256]
                    emb_ps = tp.tile([32, 256], f32, tag="small")
                    for k in range(4):
                        nc.tensor.matmul(emb_ps[:], r(hT_sb[:, k * 32:(k + 1) * 32]),
                                         r(fcW_sb[:, k * 256:(k + 1) * 256]),
                                         start=(k == 0), stop=(k == 3 and not fc_bias))
                    if fc_bias:
                        ones_r32b = tl.tile([1, 32], f32r)
                        ones_r32bf = tl.tile([1, 32], f32)
                        nc.gpsimd.memset(ones_r32bf[:], 1.0)
                        nc.vector.tensor_copy(ones_r32b[:], ones_r32bf[:])
                        nc.tensor.matmul(emb_ps[:], r(ones_r32b[:]), r(fcb_row[:]),
                                         start=False, stop=True)
                    emb_sb = tl.tile([32, 256], f32r)
                    nc.vector.tensor_copy(emb_sb[:], emb_ps[:])
                    if "emb" in taps:
                        nc.sync.dma_start(out=taps["emb"].ap(), in_=emb_sb[:])

                    # embT [128, (mo,t)]
                    embT_ps = tp.tile([128, 64], f32, tag="small")
                    for mo in (0, 1):
                        for k in range(4):
                            nc.tensor.matmul(
                                embT_ps[:, mo * 32:(mo + 1) * 32],
                                r(fcW_sb[:, k * 256 + mo * 128: k * 256 + (mo + 1) * 128]),
                                r(hT_sb[:, k * 32:(k + 1) * 32]),
                                start=(k == 0), stop=(k == 3))
                    embT_sb = tl.tile([128, 64], f32r)
                    if fc_bias:
                        for mo in (0, 1):
                            nc.scalar.activation(embT_sb[:, mo * 32:(mo + 1) * 32],
                                                 embT_ps[:, mo * 32:(mo + 1) * 32],
                                                 AF.Identity,
                                                 bias=fcb_col[:, mo:mo + 1])
                    else:
                        nc.vector.tensor_copy(embT_sb[:], embT_ps[:])

                    # attention scores [1, 32] ; softmax over free dim
                    sc_ps = tp.tile([1, 32], f32, tag="small")
                    for mo in (0, 1):
                        nc.tensor.matmul(sc_ps[:], rf(attnW_sb[:, mo:mo + 1]),
                                         rf(embT_sb[:, mo * 32:(mo + 1) * 32]),
                                         start=(mo == 0), stop=(mo == 1))
                    sc_sb = tl.tile([1, 32], f32)
                    nc.vector.tensor_copy(sc_sb[:], sc_ps[:])
                    mx = tl.tile([1, 1], f32)
                    nc.vector.tensor_reduce(mx[:], sc_sb[:], axis=mybir.AxisListType.X,
                                            op=ALU.max)
                    mxn = tl.tile([1, 1], f32)
                    nc.vector.tensor_scalar_mul(mxn[:], mx[:], -1.0)
                    ex = tl.tile([1, 32], f32)
                    ssum = tl.tile([1, 1], f32)
                    nc.scalar.activation(ex[:], sc_sb[:], AF.Exp, bias=mxn[:],
                                         accum_out=ssum[:])
                    rs = tl.tile([1, 1], f32)
                    nc.vector.reciprocal(rs[:], ssum[:])
                    w_row = tl.tile([1, 32], f32r)
                    nc.vector.tensor_scalar_mul(w_row[:], ex[:], rs[:])
                    if "w" in taps:
                        nc.sync.dma_start(out=taps["w"].ap(), in_=w_row[:])

                    # w column, x_weighted, head
                    wc_ps = tp.tile([32, 1], f32, tag="small")
                    nc.tensor.matmul(wc_ps[:], rf(w_row[:]), rf(ones_11[:]),
                                     start=True, stop=True)
                    wc_sb = tl.tile([32, 1], f32r)
                    nc.vector.tensor_copy(wc_sb[:], wc_ps[:])
                    xw_ps = tp.tile([1, 256], f32, tag="small")
                    nc.tensor.matmul(xw_ps[:], rf(wc_sb[:]), rf(emb_sb[:]),
                                     start=True, stop=True)
                    xw_row = tl.tile([1, 256], f32r)
                    nc.vector.tensor_copy(xw_row[:], xw_ps[:])
                    xwc_ps = tp.tile([128, 2], f32, tag="small")
                    for mo in (0, 1):
                        nc.tensor.matmul(xwc_ps[:, mo:mo + 1],
                                         rf(xw_row[0:1, mo * 128:(mo + 1) * 128]),
                                         rf(ones_11[:]), start=True, stop=True)
                    xwc_sb = tl.tile([128, 2], f32r)
                    nc.vector.tensor_copy(xwc_sb[:], xwc_ps[:])
                    fin_ps = tp.tile([1, 512], f32, tag="small")
                    for mo in (0, 1):
                        nc.tensor.matmul(fin_ps[:], rf(xwc_sb[:, mo:mo + 1]),
                                         rf(outW_sb[:, mo * 512:(mo + 1) * 512]),
                                         start=(mo == 0), stop=(mo == 1 and not out_bias))
                    if out_bias:
                        nc.tensor.matmul(fin_ps[:], rf(ones_11[:]), rf(outb_row[:]),
                                         start=False, stop=True)
                    fin_sb = tl.tile([1, 512], f32)
                    nc.vector.tensor_copy(fin_sb[:], fin_ps[:])
                    nc.sync.dma_start(out=out_d.ap(), in_=fin_sb[:])

    nc.compile()
    return nc


def _get_nc(flags):
    key = tuple(sorted(flags.items()))
    if key not in _CACHE:
        _CACHE[key] = _build(flags)
    return _CACHE[key]


def kernel(**inputs):
    from concourse import bass_utils

    inp = {k: np.asarray(v) for k, v in inputs.items()}
    flags = {
        "gcn_bias": any(np.any(inp[f"b{i}"]) for i in (1, 2, 3, 4)),
        "lstm_bias": any(np.any(inp[k]) for k in
                         ("b_ih_f", "b_hh_f", "b_ih_b", "b_hh_b")),
        "fc_bias": bool(np.any(inp["fc_b"])),
        "out_bias": bool(np.any(inp["out_b"])),
        "debug_taps": False,
    }
    nc = _get_nc(flags)

    base = {
        "edge_index": np.ascontiguousarray(inp["edge_index"].astype(np.int32)),
        "W1": np.ascontiguousarray(inp["W1"].astype(np.float32)),
        "W2": np.ascontiguousarray(inp["W2"].astype(np.float32)),
        "W3": np.ascontiguousarray(inp["W3"].astype(np.float32)),
        "W4": np.ascontiguousarray(inp["W4"].astype(np.float32)),
        "W_ih_f": np.ascontiguousarray(inp["W_ih_f"].astype(np.float32)),
        "W_ih_b": np.ascontiguousarray(inp["W_ih_b"].astype(np.float32)),
        "fc_W": np.ascontiguousarray(inp["fc_W"].astype(np.float32)),
        "attn_W": np.ascontiguousarray(inp["attn_W"].astype(np.float32)),
        "out_W": np.ascontiguousarray(inp["out_W"].astype(np.float32)),
    }
    if flags["gcn_bias"]:
        for i in (1, 2, 3, 4):
            base[f"b{i}"] = np.ascontiguousarray(inp[f"b{i}"].astype(np.float32))
    if flags["lstm_bias"]:
        for k in ("b_ih_f", "b_hh_f", "b_ih_b", "b_hh_b"):
            base[k] = np.ascontiguousarray(inp[k].astype(np.float32))
    if flags["fc_bias"]:
        base["fc_b"] = np.ascontiguousarray(inp["fc_b"].astype(np.float32))
    if flags["out_bias"]:
        base["out_b"] = np.ascontiguousarray(inp["out_b"].astype(np.float32))

    data = inp["data"].astype(np.float32)
    in_maps = [dict(base, data_local=np.ascontiguousarray(data[c]))
               for c in range(NCORES)]

    global LAST_RESULT
    res = bass_utils.run_bass_kernel_spmd(nc, in_maps,
                                          core_ids=list(range(NCORES)),
                                          **RUN_KWARGS)
    LAST_RESULT = res
    return np.concatenate([r["out"] for r in res.results], axis=0)


if __name__ == "__main__":
    import reference
    inputs = {k: np.asarray(v) for k, v in reference.setup_inputs().items()}
    got = kernel(**inputs)
    print(got.shape, got.dtype)



# revision 13
# speedup vs baseline: 1.1117x; 1.1117x over previous
"""Trainium2 Bass kernel for nn_DeepConvGraphEncoderPre.

Model: 4x GCN (dense normalized adjacency) -> mean-pool over nodes ->
single-step BiLSTM -> fc -> temporal attention over T -> linear head.

Sharding: data-parallel over batch B=8 across 8 NeuronCores (1 batch row
per core).  The normalized dense adjacency A^T [256,512-layout] is built
ON DEVICE from edge_index via one-hot matmuls (exact, handles duplicate
edges); self-loops are added analytically as an identity.  Every GCN
layer is two dense matmuls (aggregate-first): x <- relu((A x) W + b).

Key optimizations vs the f32r baseline:
- all GCN matmuls in bf16 (validated: final rel err ~3e-3 vs 2e-2 tol);
  every matmul streams at 1 cycle/row regardless of moving-free size.
- graph PAIRS merged into single matmuls for L1/L2 via block-diagonal
  W1/W2 (built on host), halving matmul count there.
- all weights are pre-laid-out and pre-cast on HOST (pure relayout);
  input data is host-transposed into the exact SBUF layout so the big
  DMA is 128 contiguous 8KB lines instead of 8192 x 256B descriptors.
- PSUM evacuations balanced across vector+scalar; node-pooling fused
  into relu via tensor_tensor_reduce on vector.
- LSTM tail: forget gate dropped (unused at window_size=1), sigmoid
  computed from tanh (host-folded 1/2 scales) so one activation-table
  load covers i/o/g/c; attention bias dropped (softmax shift-invariant);
  weighted sum via fused multiply-accumulate instead of extra matmuls.
"""

import numpy as np
import ml_dtypes

B, T, N, F, E = 8, 32, 256, 64, 4096
H, EMB, OUT = 256, 256, 512
NCORES = 8
NPAIR = T // 2  # graph pairs per core

_CACHE = {}
RUN_KWARGS = {}   # test harness may set {"trace": True, ...}
LAST_RESULT = None


def _build(flags):
    import concourse.mybir as mybir
    import concourse.tile as tile
    from concourse import bacc
    from concourse.masks import make_identity

    dt = mybir.dt
    f32, f32r, bf16, i32 = dt.float32, dt.float32r, dt.bfloat16, dt.int32
    AF = mybir.ActivationFunctionType
    ALU = mybir.AluOpType

    gcn_bias, lstm_bias, fc_bias, out_bias = (
        flags["gcn_bias"], flags["lstm_bias"], flags["fc_bias"], flags["out_bias"])

    nc = bacc.Bacc("TRN2", target_bir_lowering=False, debug=False,
                   num_devices=NCORES)

    def rf(ap):
        return ap.bitcast(f32r)

    # ---------------- DRAM I/O (all host-prepped layouts) ----------------
    x0_d = nc.dram_tensor("x0", [128, NPAIR * 256], bf16, kind="ExternalInput")
    edge_d = nc.dram_tensor("edge_index", [2, E], i32, kind="ExternalInput")
    W1_d = nc.dram_tensor("W1blk", [128, 128], bf16, kind="ExternalInput")
    W2_d = nc.dram_tensor("W2blk", [128, 256], bf16, kind="ExternalInput")
    W3_d = nc.dram_tensor("W3p", [128, 256], bf16, kind="ExternalInput")
    W4_d = nc.dram_tensor("W4p", [128, 512], bf16, kind="ExternalInput")
    WihT_d = nc.dram_tensor("WihTp", [128, 3072], f32r, kind="ExternalInput")
    fcW_d = nc.dram_tensor("fcWp", [128, 1024], f32r, kind="ExternalInput")
    attnW_d = nc.dram_tensor("attnWp", [128, 2], f32r, kind="ExternalInput")
    outW_d = nc.dram_tensor("outWp", [128, 1024], f32r, kind="ExternalInput")
    if gcn_bias:
        bb1_d = nc.dram_tensor("bb1", [128, 128], f32, kind="ExternalInput")
        bb2_d = nc.dram_tensor("bb2", [128, 256], f32, kind="ExternalInput")
        bb3_d = nc.dram_tensor("bb3", [128, 256], f32, kind="ExternalInput")
        b4c_d = nc.dram_tensor("b4col", [128, 2], f32, kind="ExternalInput")
    if lstm_bias:
        bihT_d = nc.dram_tensor("bihT", [1, 1536], f32r, kind="ExternalInput")
    if fc_bias:
        fcb_d = nc.dram_tensor("fcb_col", [128, 2], f32, kind="ExternalInput")
    if out_bias:
        outb_d = nc.dram_tensor("outb_row", [1, 512], f32r, kind="ExternalInput")
    out_d = nc.dram_tensor("out", [1, OUT], f32, kind="ExternalOutput")

    with tile.TileContext(nc) as tc:
        with tc.tile_pool(name="const", bufs=1) as cp:
            AT_sb = cp.tile([128, 512], bf16)       # col k*256+d ; A^T[s,d], s=k*128+p
            x0_sb = cp.tile([128, NPAIR * 256], bf16)
            W1_sb = cp.tile([128, 128], bf16)
            W2_sb = cp.tile([128, 256], bf16)
            W3_sb = cp.tile([128, 256], bf16)
            W4_sb = cp.tile([128, 512], bf16)
            WihT_sb = cp.tile([128, 3072], f32r)    # col k*1536 + g'*512 + d*256 + h
            fcW_sb = cp.tile([128, 1024], f32r)     # col k*256 + m   (pre-scaled 0.5)
            attnW_sb = cp.tile([128, 2], f32r)
            outW_sb = cp.tile([128, 1024], f32r)    # col mo*512 + o
            pooledT_sb = cp.tile([128, 64], f32r)   # col mo*32 + t
            ident = cp.tile([128, 128], f32)
            ones_col = cp.tile([128, 1], f32)
            zeros2 = cp.tile([128, 256], f32)
            if gcn_bias:
                bb1_sb = cp.tile([128, 128], f32)
                bb2_sb = cp.tile([128, 256], f32)
                bb3_sb = cp.tile([128, 256], f32)
                b4c_sb = cp.tile([128, 2], f32)
            if lstm_bias or out_bias:
                ones_f = cp.tile([1, 32], f32)
                ones_r = cp.tile([1, 32], f32r)
            if lstm_bias:
                bihT_sb = cp.tile([1, 1536], f32r)
            if fc_bias:
                fcb_sb = cp.tile([128, 2], f32)
            if out_bias:
                outb_sb = cp.tile([1, 512], f32r)

            # ---- DMA issue: sync gets the critical-path tensors, scalar
            # the weights (parallel issue on two queues) ----
            eg_sb = cp.tile([128, 64], i32)   # col j<32: src ; col 32+j: dst
            nc.sync.dma_start(
                out=eg_sb[:].rearrange("p (two j) -> p two j", two=2),
                in_=edge_d.ap().rearrange("two (p j) -> p two j", p=128))
            nc.sync.dma_start(out=x0_sb[:], in_=x0_d.ap())
            nc.sync.dma_start(out=WihT_sb[:], in_=WihT_d.ap())
            nc.sync.dma_start(out=W1_sb[:], in_=W1_d.ap())
            nc.sync.dma_start(out=W2_sb[:], in_=W2_d.ap())
            nc.sync.dma_start(out=W3_sb[:], in_=W3_d.ap())
            nc.sync.dma_start(out=W4_sb[:], in_=W4_d.ap())
            nc.sync.dma_start(out=fcW_sb[:], in_=fcW_d.ap())
            nc.sync.dma_start(out=attnW_sb[:], in_=attnW_d.ap())
            nc.sync.dma_start(out=outW_sb[:], in_=outW_d.ap())
            if gcn_bias:
                nc.sync.dma_start(out=bb1_sb[:], in_=bb1_d.ap())
                nc.sync.dma_start(out=bb2_sb[:], in_=bb2_d.ap())
                nc.sync.dma_start(out=bb3_sb[:], in_=bb3_d.ap())
                nc.sync.dma_start(out=b4c_sb[:], in_=b4c_d.ap())
            if lstm_bias:
                nc.sync.dma_start(out=bihT_sb[:], in_=bihT_d.ap())
            if fc_bias:
                nc.sync.dma_start(out=fcb_sb[:], in_=fcb_d.ap())
            if out_bias:
                nc.sync.dma_start(out=outb_sb[:], in_=outb_d.ap())

            nc.gpsimd.memset(ones_col[:], 1.0)
            nc.gpsimd.memset(zeros2[:], 0.0)
            make_identity(nc, ident[:])
            if lstm_bias or out_bias:
                nc.gpsimd.memset(ones_f[:], 1.0)
                nc.vector.tensor_copy(ones_r[:], ones_f[:])

            # ============ stage A: A^T build ============
            with (
                tc.tile_pool(name="ab_sb", bufs=2) as ab,
                tc.tile_pool(name="ab_ps", bufs=1, space="PSUM") as abp,
                tc.tile_pool(name="oh", bufs=4) as ohp,
            ):
                # iota row 0..255 (int32 -> bf16), broadcast to 128 partitions
                iota_i = ab.tile([1, 256], i32)
                nc.gpsimd.iota(iota_i[:], pattern=[[1, 256]], base=0,
                               channel_multiplier=0)
                iota_b = ab.tile([1, 256], bf16)
                nc.vector.tensor_copy(iota_b[:], iota_i[:])
                iota_bc = ab.tile([128, 256], bf16)
                nc.gpsimd.partition_broadcast(iota_bc[:], iota_b[:])
                eg_b = ab.tile([128, 64], f32)
                nc.vector.tensor_copy(eg_b[:], eg_sb[:])

                # accumulate A^T_unnorm = sum_e onehot_src^T(slice) @ onehot_dst
                # (src one-hots on vector, dst one-hots on gpsimd)
                atun_ps = abp.tile([128, 512], f32)
                for c in range(32):
                    oh_s = ohp.tile([128, 256], bf16, tag="ohs")
                    nc.vector.tensor_scalar(oh_s[:], iota_bc[:],
                                            eg_b[:, c:c + 1], None,
                                            op0=ALU.is_equal)
                    oh_d = ohp.tile([128, 256], bf16, tag="ohd")
                    nc.vector.tensor_scalar(oh_d[:], iota_bc[:],
                                            eg_b[:, 32 + c:33 + c], None,
                                            op0=ALU.is_equal)
                    for m in (0, 1):
                        nc.tensor.matmul(atun_ps[:, m * 256:(m + 1) * 256],
                                         oh_s[:, m * 128:(m + 1) * 128], oh_d[:],
                                         start=(c == 0 and m == 0),
                                         stop=(c == 31 and m == 1))
                atun_sb = ab.tile([128, 512], f32r)
                nc.vector.tensor_copy(atun_sb[:], atun_ps[:])
                # self-loops: += I on the diagonal (s = m*128+p, d = s)
                for m in (0, 1):
                    sl = atun_sb[:, m * 256 + m * 128: m * 256 + (m + 1) * 128]
                    nc.vector.tensor_add(sl, sl, ident[:])

                # deg (row + col forms), dinv = 1/sqrt(deg)   (deg >= 1 always)
                deg_ps = abp.tile([1, 256], f32, tag="deg")
                for m in (0, 1):
                    nc.tensor.matmul(deg_ps[:], ones_col[:],
                                     atun_sb[:, m * 256:(m + 1) * 256].bitcast(f32),
                                     start=(m == 0), stop=(m == 1))
                degc_ps = abp.tile([128, 2], f32, tag="degc")
                for dm in (0, 1):
                    for m in (0, 1):
                        nc.tensor.matmul(
                            degc_ps[:, dm:dm + 1],
                            atun_sb[:, m * 256 + dm * 128:
                                    m * 256 + (dm + 1) * 128].bitcast(f32),
                            ones_col[:], start=(m == 0), stop=(m == 1))
                dinv_row = ab.tile([1, 256], f32)
                nc.vector.reciprocal(dinv_row[:], deg_ps[:])
                nc.scalar.sqrt(dinv_row[:], dinv_row[:])
                dinv_col = ab.tile([128, 2], f32)
                nc.vector.reciprocal(dinv_col[:], degc_ps[:])
                nc.scalar.sqrt(dinv_col[:], dinv_col[:])
                dinv_bc = ab.tile([128, 256], f32)
                nc.gpsimd.partition_broadcast(dinv_bc[:], dinv_row[:])

                # AT[s,d] = dinv[s] * ATun[s,d] * dinv[d]  (bf16 out)
                for m in (0, 1):
                    nc.vector.scalar_tensor_tensor(
                        out=AT_sb[:, m * 256:(m + 1) * 256],
                        in0=atun_sb[:, m * 256:(m + 1) * 256],
                        scalar=dinv_col[:, m:m + 1],
                        in1=dinv_bc[:],
                        op0=ALU.mult, op1=ALU.mult)

            # ================= stage B: GCN loop (graph pairs) =================
            with (
                tc.tile_pool(name="work", bufs=2) as wk,
                tc.tile_pool(name="psA", bufs=2, space="PSUM") as psA,
                tc.tile_pool(name="psB", bufs=2, space="PSUM") as psB,
                tc.tile_pool(name="psC", bufs=2, space="PSUM") as psC,
            ):
                for j in range(NPAIR):
                    xb = j * 256
                    # ---- L1 ----  agg over both graphs at once:
                    # stat x0[s-block, (g,c)=128], moving AT k-slice
                    agg1 = psC.tile([128, 256], f32, tag="C")
                    for k in (0, 1):
                        nc.tensor.matmul(agg1[:],
                                         x0_sb[:, xb + k * 128: xb + (k + 1) * 128],
                                         AT_sb[:, k * 256:(k + 1) * 256],
                                         start=(k == 0), stop=(k == 1))
                    agg1_sb = wk.tile([128, 256], bf16, tag="agg1")
                    nc.vector.tensor_copy(agg1_sb[:], agg1[:])
                    # W step: stat agg1[(g,c), d-slice], moving block-diag W1
                    z1 = psC.tile([128, 256], f32, tag="C")
                    for m in (0, 1):
                        nc.tensor.matmul(z1[:, m * 128:(m + 1) * 128],
                                         agg1_sb[:, m * 128:(m + 1) * 128],
                                         W1_sb[:], start=True, stop=True)
                    x1 = wk.tile([128, 256], bf16, tag="x1")
                    if gcn_bias:
                        z1b = wk.tile([128, 256], f32, tag="z1b")
                        nc.vector.tensor_add(
                            z1b[:].rearrange("p (m q) -> p m q", m=2),
                            z1[:].rearrange("p (m q) -> p m q", m=2),
                            bb1_sb[:].rearrange("p q -> p 1 q").broadcast_to([128, 2, 128]))
                        nc.scalar.activation(x1[:], z1b[:], AF.Relu)
                    else:
                        nc.scalar.activation(x1[:], z1[:], AF.Relu)

                    # ---- L2 ----
                    agg2 = psC.tile([128, 256], f32, tag="C")
                    for k in (0, 1):
                        nc.tensor.matmul(agg2[:],
                                         x1[:, k * 128:(k + 1) * 128],
                                         AT_sb[:, k * 256:(k + 1) * 256],
                                         start=(k == 0), stop=(k == 1))
                    agg2_sb = wk.tile([128, 256], bf16, tag="agg2")
                    nc.vector.tensor_copy(agg2_sb[:], agg2[:])
                    z2 = psB.tile([128, 512], f32, tag="B")
                    for m in (0, 1):
                        nc.tensor.matmul(z2[:, m * 256:(m + 1) * 256],
                                         agg2_sb[:, m * 128:(m + 1) * 128],
                                         W2_sb[:], start=True, stop=True)
                    x2 = wk.tile([128, 512], bf16, tag="x2")
                    if gcn_bias:
                        z2b = wk.tile([128, 512], f32, tag="z2b")
                        nc.vector.tensor_add(
                            z2b[:].rearrange("p (m q) -> p m q", m=2),
                            z2[:].rearrange("p (m q) -> p m q", m=2),
                            bb2_sb[:].rearrange("p q -> p 1 q").broadcast_to([128, 2, 256]))
                        nc.scalar.activation(x2[:], z2b[:], AF.Relu)
                    else:
                        nc.scalar.activation(x2[:], z2[:], AF.Relu)

                    # ---- L3 ----
                    agg3 = psB.tile([128, 512], f32, tag="B")
                    for g in (0, 1):
                        for k in (0, 1):
                            nc.tensor.matmul(
                                agg3[:, g * 256:(g + 1) * 256],
                                x2[:, k * 256 + g * 128: k * 256 + (g + 1) * 128],
                                AT_sb[:, k * 256:(k + 1) * 256],
                                start=(k == 0), stop=(k == 1))
                    agg3_sb = wk.tile([128, 512], bf16, tag="agg3")
                    nc.vector.tensor_copy(agg3_sb[:], agg3[:])
                    z3 = psA.tile([128, 1024], f32, tag="A")
                    for g in (0, 1):
                        for m in (0, 1):
                            nc.tensor.matmul(
                                z3[:, g * 512 + m * 256: g * 512 + (m + 1) * 256],
                                agg3_sb[:, g * 256 + m * 128: g * 256 + (m + 1) * 128],
                                W3_sb[:], start=True, stop=True)
                    x3 = wk.tile([128, 1024], bf16, tag="x3")
                    if gcn_bias:
                        z3b = wk.tile([128, 1024], f32, tag="z3b")
                        nc.vector.tensor_add(
                            z3b[:].rearrange("p (gm q) -> p gm q", gm=4),
                            z3[:].rearrange("p (gm q) -> p gm q", gm=4),
                            bb3_sb[:].rearrange("p q -> p 1 q").broadcast_to([128, 4, 256]))
                        nc.scalar.activation(x3[:], z3b[:], AF.Relu)
                    else:
                        nc.scalar.activation(x3[:], z3[:], AF.Relu)

                    # ---- L4 ----
                    agg4 = psA.tile([128, 1024], f32, tag="A")
                    for g in (0, 1):
                        for mc in (0, 1):
                            for k in (0, 1):
                                nc.tensor.matmul(
                                    agg4[:, g * 512 + mc * 256: g * 512 + (mc + 1) * 256],
                                    x3[:, g * 512 + k * 256 + mc * 128:
                                          g * 512 + k * 256 + (mc + 1) * 128],
                                    AT_sb[:, k * 256:(k + 1) * 256],
                                    start=(k == 0), stop=(k == 1))
                    agg4_sb = wk.tile([128, 1024], bf16, tag="agg4")
                    nc.scalar.copy(agg4_sb[:], agg4[:])
                    z4 = psA.tile([128, 1024], f32, tag="A")
                    for g in (0, 1):
                        for mo in (0, 1):
                            for k in (0, 1):
                                nc.tensor.matmul(
                                    z4[:, g * 512 + mo * 256: g * 512 + (mo + 1) * 256],
                                    W4_sb[:, k * 256 + mo * 128: k * 256 + (mo + 1) * 128],
                                    agg4_sb[:, g * 512 + k * 256: g * 512 + (k + 1) * 256],
                                    start=(k == 0), stop=(k == 1))
                    # relu + mean-pool over nodes (free axis); 1/N folded into WihT
                    x4 = wk.tile([128, 1024], bf16, tag="x4")
                    with nc.allow_low_precision(reason="f32r pool accum (32-bit)"):
                        for g in (0, 1):
                            t_idx = 2 * j + g
                            for mo in (0, 1):
                                sl = slice(g * 512 + mo * 256, g * 512 + (mo + 1) * 256)
                                nc.scalar.activation(
                                    x4[:, sl], z4[:, sl], AF.Relu,
                                    bias=(b4c_sb[:, mo:mo + 1] if gcn_bias else 0.0),
                                    accum_out=pooledT_sb[:, mo * 32 + t_idx:
                                                         mo * 32 + t_idx + 1])

            # ======= stage C: LSTM + fc + attention + head =======
            # gates laid out (i, o, g) x (dir) x h; i,o have 0.5 folded into
            # WihT so sigmoid(x) = 0.5*(1+tanh(x/2)) needs only tanh.
            with (
                tc.tile_pool(name="tail", bufs=1) as tl,
                tc.tile_pool(name="tailps_g", bufs=1, space="PSUM") as tpg,
                tc.tile_pool(name="tailps", bufs=2, space="PSUM") as tp,
            ):
                g_ps = tpg.tile([32, 1536], f32, tag="gates")
                for s in range(3):
                    for k in (0, 1):
                        nc.tensor.matmul(
                            g_ps[:, s * 512:(s + 1) * 512],
                            rf(pooledT_sb[:, k * 32:(k + 1) * 32]),
                            WihT_sb[:, k * 1536 + s * 512: k * 1536 + (s + 1) * 512],
                            start=(k == 0),
                            stop=(k == 1 and not lstm_bias))
                    if lstm_bias:
                        nc.tensor.matmul(g_ps[:, s * 512:(s + 1) * 512],
                                         ones_r[:],
                                         bihT_sb[:, s * 512:(s + 1) * 512],
                                         start=False, stop=True)
                th = tl.tile([32, 1536], f32)
                nc.scalar.activation(th[:], g_ps[:], AF.Tanh)
                # c2 = 2c = (1+tanh(i/2))*tanh(g) ; tc = tanh(c2 * 0.5)
                c2 = tl.tile([32, 512], f32)
                nc.vector.scalar_tensor_tensor(
                    out=c2[:], in0=th[:, 0:512], scalar=1.0,
                    in1=th[:, 1024:1536], op0=ALU.add, op1=ALU.mult)
                tc_sb = tl.tile([32, 512], f32)
                nc.scalar.activation(tc_sb[:], c2[:], AF.Tanh, scale=0.5)
                # h2 = 2h = (1+tanh(o/2))*tanh(c); the remaining 1/2 is folded
                # into fcW (pre-scaled 0.5 on host)
                h2 = tl.tile([32, 512], f32)
                nc.vector.scalar_tensor_tensor(
                    out=h2[:], in0=th[:, 512:1024], scalar=1.0,
                    in1=tc_sb[:], op0=ALU.add, op1=ALU.mult)

                # transpose h2 -> hT [128, (k,t)]
                hT_ps = tp.tile([128, 128], f32, tag="small")
                for k in range(4):
                    nc.tensor.transpose(hT_ps[:, k * 32:(k + 1) * 32],
                                        h2[:, k * 128:(k + 1) * 128],
                                        ident[0:32, 0:32])
                hT_sb = tl.tile([128, 128], f32r)
                nc.vector.tensor_copy(hT_sb[:], hT_ps[:])

                # embT [128, (mo,t)] = fcW^T @ hT
                embT_ps = tp.tile([128, 64], f32, tag="small")
                for mo in (0, 1):
                    for k in range(4):
                        nc.tensor.matmul(
                            embT_ps[:, mo * 32:(mo + 1) * 32],
                            fcW_sb[:, k * 256 + mo * 128: k * 256 + (mo + 1) * 128],
                            hT_sb[:, k * 32:(k + 1) * 32],
                            start=(k == 0), stop=(k == 3))
                embT_sb = tl.tile([128, 64], f32r)
                if fc_bias:
                    for mo in (0, 1):
                        nc.scalar.activation(embT_sb[:, mo * 32:(mo + 1) * 32],
                                             embT_ps[:, mo * 32:(mo + 1) * 32],
                                             AF.Identity,
                                             bias=fcb_sb[:, mo:mo + 1])
                else:
                    nc.vector.tensor_copy(embT_sb[:], embT_ps[:])

                # attention scores [1, 32]; attn_b dropped (softmax shift-inv);
                # no max-subtract (scores are O(1) by construction)
                sc_ps = tp.tile([1, 32], f32, tag="small")
                for mo in (0, 1):
                    nc.tensor.matmul(sc_ps[:], attnW_sb[:, mo:mo + 1],
                                     embT_sb[:, mo * 32:(mo + 1) * 32],
                                     start=(mo == 0), stop=(mo == 1))
                ex = tl.tile([1, 32], f32)
                ssum = tl.tile([1, 1], f32)
                nc.scalar.activation(ex[:], sc_ps[:], AF.Exp, accum_out=ssum[:])
                rs = tl.tile([1, 1], f32)
                nc.vector.reciprocal(rs[:], ssum[:])
                w_row = tl.tile([1, 32], f32)
                nc.vector.tensor_scalar_mul(w_row[:], ex[:], rs[:])
                w_bc = tl.tile([128, 32], f32)
                nc.gpsimd.partition_broadcast(w_bc[:], w_row[:])

                # x_weighted[m] = sum_t embT[m,t] * w[t]  (fused mul+accum)
                xw_scr = tl.tile([128, 64], f32r)
                xw_col = tl.tile([128, 2], f32r)
                with nc.allow_low_precision(reason="f32r weighted-sum accum"):
                    for mo in (0, 1):
                        nc.vector.scalar_tensor_tensor(
                            out=xw_scr[:, mo * 32:(mo + 1) * 32],
                            in0=embT_sb[:, mo * 32:(mo + 1) * 32], scalar=1.0,
                            in1=w_bc[:], op0=ALU.mult, op1=ALU.mult,
                            accum_out=xw_col[:, mo:mo + 1])

                # head: out = xw @ out_W (+ out_b)
                fin_ps = tp.tile([1, 512], f32, tag="small")
                for mo in (0, 1):
                    nc.tensor.matmul(fin_ps[:], xw_col[:, mo:mo + 1],
                                     outW_sb[:, mo * 512:(mo + 1) * 512],
                                     start=(mo == 0),
                                     stop=(mo == 1 and not out_bias))
                if out_bias:
                    nc.tensor.matmul(fin_ps[:], ones_r[0:1, 0:1], outb_sb[:],
                                     start=False, stop=True)
                fin_sb = tl.tile([1, 512], f32)
                nc.vector.tensor_copy(fin_sb[:], fin_ps[:])
                nc.sync.dma_start(out=out_d.ap(), in_=fin_sb[:])

    nc.compile()
    return nc


def _get_nc(flags):
    key = tuple(sorted(flags.items()))
    if key not in _CACHE:
        _CACHE[key] = _build(flags)
    return _CACHE[key]


def kernel(**inputs):
    from concourse import bass_utils

    bf = ml_dtypes.bfloat16
    inp = {k: np.asarray(v) for k, v in inputs.items()}
    flags = {
        "gcn_bias": any(np.any(inp[f"b{i}"]) for i in (1, 2, 3, 4)),
        "lstm_bias": any(np.any(inp[k]) for k in
                         ("b_ih_f", "b_hh_f", "b_ih_b", "b_hh_b")),
        "fc_bias": bool(np.any(inp["fc_b"])),
        "out_bias": bool(np.any(inp["out_b"])),
    }
    nc = _get_nc(flags)

    f32 = np.float32
    W1 = inp["W1"].astype(f32)
    W2 = inp["W2"].astype(f32)
    W1blk = np.zeros((128, 128), f32)
    W1blk[:64, :64] = W1
    W1blk[64:, 64:] = W1
    W2blk = np.zeros((128, 256), f32)
    W2blk[:64, :128] = W2
    W2blk[64:, 128:] = W2
    W4p = inp["W4"].astype(f32).reshape(2, 128, 256).transpose(1, 0, 2).reshape(128, 512)

    # WihT: [co, g'*512 + d*256 + h], gate order (i, o, g); i,o scaled 0.5
    # (sigmoid-from-tanh), everything scaled 1/N (mean-pool folded in)
    M = np.zeros((256, 1536), f32)
    for di, dname in enumerate(("f", "b")):
        Wih = inp[f"W_ih_{dname}"].astype(f32)  # [4H, H] rows gate*256+h
        for gdst, (gsrc, sc) in enumerate([(0, 0.5), (3, 0.5), (2, 1.0)]):
            M[:, gdst * 512 + di * 256: gdst * 512 + (di + 1) * 256] = \
                Wih[gsrc * 256:(gsrc + 1) * 256, :].T * (sc / N)
    WihTp = M.reshape(2, 128, 1536).transpose(1, 0, 2).reshape(128, 3072)

    fcWp = (inp["fc_W"].astype(f32) * 0.5).reshape(4, 128, 256) \
        .transpose(1, 0, 2).reshape(128, 1024)
    attnWp = np.ascontiguousarray(inp["attn_W"].astype(f32).reshape(2, 128).T)
    outWp = inp["out_W"].astype(f32).reshape(2, 128, 512) \
        .transpose(1, 0, 2).reshape(128, 1024)

    base = {
        "edge_index": np.ascontiguousarray(inp["edge_index"].astype(np.int32)),
        "W1blk": np.ascontiguousarray(W1blk.astype(bf)),
        "W2blk": np.ascontiguousarray(W2blk.astype(bf)),
        "W3p": np.ascontiguousarray(inp["W3"].astype(f32).astype(bf)),
        "W4p": np.ascontiguousarray(W4p.astype(bf)),
        "WihTp": np.ascontiguousarray(WihTp),
        "fcWp": np.ascontiguousarray(fcWp),
        "attnWp": attnWp,
        "outWp": np.ascontiguousarray(outWp),
    }
    if flags["gcn_bias"]:
        b1 = inp["b1"].astype(f32)
        b2 = inp["b2"].astype(f32)
        b3 = inp["b3"].astype(f32)
        b4 = inp["b4"].astype(f32)
        base["bb1"] = np.ascontiguousarray(
            np.tile(np.concatenate([b1, b1]), (128, 1)))
        base["bb2"] = np.ascontiguousarray(
            np.tile(np.concatenate([b2, b2]), (128, 1)))
        base["bb3"] = np.ascontiguousarray(np.tile(b3, (128, 1)))
        base["b4col"] = np.ascontiguousarray(b4.reshape(2, 128).T)
    if flags["lstm_bias"]:
        bihT = np.zeros((1, 1536), f32)
        for di, dname in enumerate(("f", "b")):
            bsum = (inp[f"b_ih_{dname}"] + inp[f"b_hh_{dname}"]).astype(f32)
            for gdst, (gsrc, sc) in enumerate([(0, 0.5), (3, 0.5), (2, 1.0)]):
                bihT[0, gdst * 512 + di * 256: gdst * 512 + (di + 1) * 256] = \
                    bsum[gsrc * 256:(gsrc + 1) * 256] * sc
        base["bihT"] = bihT
    if flags["fc_bias"]:
        base["fcb_col"] = np.ascontiguousarray(
            inp["fc_b"].astype(f32).reshape(2, 128).T)
    if flags["out_bias"]:
        base["outb_row"] = np.ascontiguousarray(
            inp["out_b"].astype(f32).reshape(1, 512))

    # x0: [p, j*256 + k*128 + g*64 + c] = data[2j+g, k*128+p, c], bf16
    data = inp["data"].astype(f32)
    in_maps = []
    for c in range(NCORES):
        v = data[c].reshape(NPAIR, 2, 2, 128, F)          # [j, g, k, p, c]
        x0 = v.transpose(3, 0, 2, 1, 4).reshape(128, NPAIR * 256)
        in_maps.append(dict(base, x0=np.ascontiguousarray(x0.astype(bf))))

    global LAST_RESULT
    res = bass_utils.run_bass_kernel_spmd(nc, in_maps,
                                          core_ids=list(range(NCORES)),
                                          **RUN_KWARGS)
    LAST_RESULT = res
    return np.concatenate([r["out"] for r in res.results], axis=0)


if __name__ == "__main__":
    import reference
    inputs = {k: np.asarray(v) for k, v in reference.setup_inputs().items()}
    got = kernel(**inputs)
    print(got.shape, got.dtype)


# revision 20
# speedup vs baseline: 1.1895x; 1.0700x over previous
"""Trainium2 Bass kernel for nn_DeepConvGraphEncoderPre.

Model: 4x GCN (dense normalized adjacency) -> mean-pool over nodes ->
single-step BiLSTM -> fc -> temporal attention over T -> linear head.

Sharding: data-parallel over batch B=8 across 8 NeuronCores (1 batch row
per core).  The normalized dense adjacency A^T [256,512-layout] is built
ON DEVICE from edge_index via one-hot matmuls (exact, handles duplicate
edges); self-loops are added analytically as an identity.  Every GCN
layer is two dense matmuls (aggregate-first): x <- relu((A x) W + b).

Key optimizations vs the f32r baseline:
- all GCN matmuls in bf16 (validated: final rel err ~3e-3 vs 2e-2 tol);
  every matmul streams at 1 cycle/row regardless of moving-free size.
- graph PAIRS merged into single matmuls for L1/L2 via block-diagonal
  W1/W2 (built on host), halving matmul count there.
- all weights are pre-laid-out and pre-cast on HOST (pure relayout);
  input data is host-transposed into the exact SBUF layout so the big
  DMA is 128 contiguous 8KB lines instead of 8192 x 256B descriptors.
- PSUM evacuations balanced across vector+scalar; node-pooling fused
  into relu via tensor_tensor_reduce on vector.
- LSTM tail: forget gate dropped (unused at window_size=1), sigmoid
  computed from tanh (host-folded 1/2 scales) so one activation-table
  load covers i/o/g/c; attention bias dropped (softmax shift-invariant);
  weighted sum via fused multiply-accumulate instead of extra matmuls.
"""

import numpy as np
import ml_dtypes

B, T, N, F, E = 8, 32, 256, 64, 4096
H, EMB, OUT = 256, 256, 512
NCORES = 8
NPAIR = T // 2  # graph pairs per core

_CACHE = {}
RUN_KWARGS = {}   # test harness may set {"trace": True, ...}
LAST_RESULT = None


def _build(flags):
    import concourse.mybir as mybir
    import concourse.tile as tile
    from concourse import bacc
    from concourse.masks import make_identity

    dt = mybir.dt
    f32, f32r, bf16, i32 = dt.float32, dt.float32r, dt.bfloat16, dt.int32
    AF = mybir.ActivationFunctionType
    ALU = mybir.AluOpType

    gcn_bias, lstm_bias, fc_bias, out_bias = (
        flags["gcn_bias"], flags["lstm_bias"], flags["fc_bias"], flags["out_bias"])

    nc = bacc.Bacc("TRN2", target_bir_lowering=False, debug=False,
                   num_devices=NCORES)

    def rf(ap):
        return ap.bitcast(f32r)

    # ---------------- DRAM I/O (all host-prepped layouts) ----------------
    x0_d = nc.dram_tensor("x0", [128, NPAIR * 256], bf16, kind="ExternalInput")
    edge_d = nc.dram_tensor("edge_index", [2, E], i32, kind="ExternalInput")
    W1_d = nc.dram_tensor("W1blk", [128, 128], bf16, kind="ExternalInput")
    W2_d = nc.dram_tensor("W2blk", [128, 256], bf16, kind="ExternalInput")
    W3_d = nc.dram_tensor("W3p", [128, 256], bf16, kind="ExternalInput")
    W4_d = nc.dram_tensor("W4p", [128, 512], bf16, kind="ExternalInput")
    WihT_d = nc.dram_tensor("WihTp", [128, 3072], f32r, kind="ExternalInput")
    fcW_d = nc.dram_tensor("fcWp", [128, 1024], f32r, kind="ExternalInput")
    attnW_d = nc.dram_tensor("attnWp", [128, 2], f32r, kind="ExternalInput")
    outW_d = nc.dram_tensor("outWp", [128, 1024], f32r, kind="ExternalInput")
    if gcn_bias:
        bb1_d = nc.dram_tensor("bb1", [128, 128], f32, kind="ExternalInput")
        bb2_d = nc.dram_tensor("bb2", [128, 256], f32, kind="ExternalInput")
        bb3_d = nc.dram_tensor("bb3", [128, 256], f32, kind="ExternalInput")
        b4c_d = nc.dram_tensor("b4col", [128, 2], f32, kind="ExternalInput")
    if lstm_bias:
        bihT_d = nc.dram_tensor("bihT", [1, 1536], f32r, kind="ExternalInput")
    if fc_bias:
        fcb_d = nc.dram_tensor("fcb_col", [128, 2], f32, kind="ExternalInput")
    if out_bias:
        outb_d = nc.dram_tensor("outb_row", [1, 512], f32r, kind="ExternalInput")
    out_d = nc.dram_tensor("out", [1, OUT], f32, kind="ExternalOutput")

    with tile.TileContext(nc) as tc:
        with tc.tile_pool(name="const", bufs=1) as cp:
            AT_sb = cp.tile([128, 512], bf16)       # col k*256+d ; A^T[s,d], s=k*128+p
            x0_sb = cp.tile([128, NPAIR * 256], bf16)
            W1_sb = cp.tile([128, 128], bf16)
            W2_sb = cp.tile([128, 256], bf16)
            W3_sb = cp.tile([128, 256], bf16)
            W4_sb = cp.tile([128, 512], bf16)
            WihT_sb = cp.tile([128, 3072], f32r)    # col k*1536 + g'*512 + d*256 + h
            fcW_sb = cp.tile([128, 1024], f32r)     # col k*256 + m   (pre-scaled 0.5)
            attnW_sb = cp.tile([128, 2], f32r)
            outW_sb = cp.tile([128, 1024], f32r)    # col mo*512 + o
            pooledT_sb = cp.tile([128, 64], f32r)   # col mo*32 + t
            ident = cp.tile([128, 128], f32)
            ones_col = cp.tile([128, 1], f32)
            ones_row = cp.tile([1, 128], f32)
            if gcn_bias:
                bb1_sb = cp.tile([128, 128], f32)
                bb2_sb = cp.tile([128, 256], f32)
                bb3_sb = cp.tile([128, 256], f32)
                b4c_sb = cp.tile([128, 2], f32)
            if lstm_bias or out_bias:
                ones_f = cp.tile([1, 32], f32)
                ones_r = cp.tile([1, 32], f32r)
            if lstm_bias:
                bihT_sb = cp.tile([1, 1536], f32r)
            if fc_bias:
                fcb_sb = cp.tile([128, 2], f32)
            if out_bias:
                outb_sb = cp.tile([1, 512], f32r)

            # ---- DMA issue: sync gets the critical-path tensors, scalar
            # the weights (parallel issue on two queues) ----
            eg_sb = cp.tile([128, 64], i32)   # col j<32: src ; col 32+j: dst
            nc.sync.dma_start(
                out=eg_sb[:].rearrange("p (two j) -> p two j", two=2),
                in_=edge_d.ap().rearrange("two (p j) -> p two j", p=128))
            nc.sync.dma_start(out=x0_sb[:], in_=x0_d.ap())
            nc.sync.dma_start(out=WihT_sb[:], in_=WihT_d.ap())
            nc.sync.dma_start(out=W1_sb[:], in_=W1_d.ap())
            nc.sync.dma_start(out=W2_sb[:], in_=W2_d.ap())
            nc.sync.dma_start(out=W3_sb[:], in_=W3_d.ap())
            nc.sync.dma_start(out=W4_sb[:], in_=W4_d.ap())
            nc.sync.dma_start(out=fcW_sb[:], in_=fcW_d.ap())
            nc.sync.dma_start(out=attnW_sb[:], in_=attnW_d.ap())
            nc.sync.dma_start(out=outW_sb[:], in_=outW_d.ap())
            if gcn_bias:
                nc.sync.dma_start(out=bb1_sb[:], in_=bb1_d.ap())
                nc.sync.dma_start(out=bb2_sb[:], in_=bb2_d.ap())
                nc.sync.dma_start(out=bb3_sb[:], in_=bb3_d.ap())
                nc.sync.dma_start(out=b4c_sb[:], in_=b4c_d.ap())
            if lstm_bias:
                nc.sync.dma_start(out=bihT_sb[:], in_=bihT_d.ap())
            if fc_bias:
                nc.sync.dma_start(out=fcb_sb[:], in_=fcb_d.ap())
            if out_bias:
                nc.sync.dma_start(out=outb_sb[:], in_=outb_d.ap())

            nc.gpsimd.memset(ones_col[:], 1.0)
            nc.gpsimd.memset(ones_row[:], 1.0)
            make_identity(nc, ident[:])
            if lstm_bias or out_bias:
                nc.gpsimd.memset(ones_f[:], 1.0)
                nc.vector.tensor_copy(ones_r[:], ones_f[:])

            # ============ stage A: A^T build ============
            with (
                tc.tile_pool(name="ab_sb", bufs=2) as ab,
                tc.tile_pool(name="ab_ps", bufs=1, space="PSUM") as abp,
                tc.tile_pool(name="oh", bufs=4) as ohp,
            ):
                # iota 0..255 on every partition (channel_multiplier=0), cast bf16
                iota_i = ab.tile([128, 256], i32)
                nc.gpsimd.iota(iota_i[:], pattern=[[1, 256]], base=0,
                               channel_multiplier=0)
                iota_bc = ab.tile([128, 256], bf16)
                nc.vector.tensor_copy(iota_bc[:], iota_i[:])
                eg_b = ab.tile([128, 64], f32)
                nc.vector.tensor_copy(eg_b[:], eg_sb[:])

                # accumulate A^T_unnorm = sum_e onehot_src^T(slice) @ onehot_dst
                # (src one-hots on vector, dst one-hots on gpsimd)
                atun_ps = abp.tile([128, 512], f32)
                for c in range(32):
                    oh_s = ohp.tile([128, 256], bf16, tag="ohs")
                    nc.vector.tensor_scalar(oh_s[:], iota_bc[:],
                                            eg_b[:, c:c + 1], None,
                                            op0=ALU.is_equal)
                    oh_d = ohp.tile([128, 256], bf16, tag="ohd")
                    nc.vector.tensor_scalar(oh_d[:], iota_bc[:],
                                            eg_b[:, 32 + c:33 + c], None,
                                            op0=ALU.is_equal)
                    for m in (0, 1):
                        nc.tensor.matmul(atun_ps[:, m * 256:(m + 1) * 256],
                                         oh_s[:, m * 128:(m + 1) * 128], oh_d[:],
                                         start=(c == 0 and m == 0),
                                         stop=(c == 31 and m == 1))
                atun_sb = ab.tile([128, 512], f32r)
                nc.vector.tensor_copy(atun_sb[:], atun_ps[:])
                # self-loops: += I on the diagonal (s = m*128+p, d = s)
                for m in (0, 1):
                    sl = atun_sb[:, m * 256 + m * 128: m * 256 + (m + 1) * 128]
                    nc.vector.tensor_add(sl, sl, ident[:])

                # deg (row + col forms), dinv = 1/sqrt(deg)   (deg >= 1 always)
                deg_ps = abp.tile([1, 256], f32, tag="deg")
                for m in (0, 1):
                    nc.tensor.matmul(deg_ps[:], ones_col[:],
                                     atun_sb[:, m * 256:(m + 1) * 256].bitcast(f32),
                                     start=(m == 0), stop=(m == 1))
                degc_ps = abp.tile([128, 2], f32, tag="degc")
                for dm in (0, 1):
                    for m in (0, 1):
                        nc.tensor.matmul(
                            degc_ps[:, dm:dm + 1],
                            atun_sb[:, m * 256 + dm * 128:
                                    m * 256 + (dm + 1) * 128].bitcast(f32),
                            ones_col[:], start=(m == 0), stop=(m == 1))
                dinv_row = ab.tile([1, 256], f32)
                nc.vector.reciprocal(dinv_row[:], deg_ps[:])
                nc.scalar.sqrt(dinv_row[:], dinv_row[:])
                dinv_col = ab.tile([128, 2], f32)
                nc.vector.reciprocal(dinv_col[:], degc_ps[:])
                nc.scalar.sqrt(dinv_col[:], dinv_col[:])
                # broadcast dinv to all partitions via rank-1 PE matmul
                dinv_bc = abp.tile([128, 256], f32, tag="dbc")
                nc.tensor.matmul(dinv_bc[:], ones_row[:], dinv_row[:],
                                 start=True, stop=True)

                # AT[s,d] = dinv[s] * ATun[s,d] * dinv[d]  (bf16 out)
                for m in (0, 1):
                    nc.vector.scalar_tensor_tensor(
                        out=AT_sb[:, m * 256:(m + 1) * 256],
                        in0=atun_sb[:, m * 256:(m + 1) * 256],
                        scalar=dinv_col[:, m:m + 1],
                        in1=dinv_bc[:],
                        op0=ALU.mult, op1=ALU.mult)

            # ================= stage B: GCN loop (graph pairs) =================
            with (
                tc.tile_pool(name="work", bufs=2) as wk,
                tc.tile_pool(name="psA", bufs=2, space="PSUM") as psA,
                tc.tile_pool(name="psB", bufs=2, space="PSUM") as psB,
                tc.tile_pool(name="psC", bufs=2, space="PSUM") as psC,
            ):
                for j in range(NPAIR):
                    xb = j * 256
                    # ---- L1 ----  agg over both graphs at once:
                    # stat x0[s-block, (g,c)=128], moving AT k-slice
                    agg1 = psC.tile([128, 256], f32, tag="C")
                    for k in (0, 1):
                        nc.tensor.matmul(agg1[:],
                                         x0_sb[:, xb + k * 128: xb + (k + 1) * 128],
                                         AT_sb[:, k * 256:(k + 1) * 256],
                                         start=(k == 0), stop=(k == 1))
                    agg1_sb = wk.tile([128, 256], bf16, tag="agg1")
                    nc.vector.tensor_copy(agg1_sb[:], agg1[:])
                    # W step: stat agg1[(g,c), d-slice], moving block-diag W1
                    z1 = psC.tile([128, 256], f32, tag="C")
                    for m in (0, 1):
                        nc.tensor.matmul(z1[:, m * 128:(m + 1) * 128],
                                         agg1_sb[:, m * 128:(m + 1) * 128],
                                         W1_sb[:], start=True, stop=True)
                    x1 = wk.tile([128, 256], bf16, tag="x1")
                    if gcn_bias:
                        z1b = wk.tile([128, 256], f32, tag="z1b")
                        nc.vector.tensor_add(
                            z1b[:].rearrange("p (m q) -> p m q", m=2),
                            z1[:].rearrange("p (m q) -> p m q", m=2),
                            bb1_sb[:].rearrange("p q -> p 1 q").broadcast_to([128, 2, 128]))
                        nc.scalar.activation(x1[:], z1b[:], AF.Relu)
                    else:
                        nc.scalar.activation(x1[:], z1[:], AF.Relu)

                    # ---- L2 ----
                    agg2 = psC.tile([128, 256], f32, tag="C")
                    for k in (0, 1):
                        nc.tensor.matmul(agg2[:],
                                         x1[:, k * 128:(k + 1) * 128],
                                         AT_sb[:, k * 256:(k + 1) * 256],
                                         start=(k == 0), stop=(k == 1))
                    agg2_sb = wk.tile([128, 256], bf16, tag="agg2")
                    nc.vector.tensor_copy(agg2_sb[:], agg2[:])
                    z2 = psB.tile([128, 512], f32, tag="B")
                    for m in (0, 1):
                        nc.tensor.matmul(z2[:, m * 256:(m + 1) * 256],
                                         agg2_sb[:, m * 128:(m + 1) * 128],
                                         W2_sb[:], start=True, stop=True)
                    x2 = wk.tile([128, 512], bf16, tag="x2")
                    if gcn_bias:
                        z2b = wk.tile([128, 512], f32, tag="z2b")
                        nc.vector.tensor_add(
                            z2b[:].rearrange("p (m q) -> p m q", m=2),
                            z2[:].rearrange("p (m q) -> p m q", m=2),
                            bb2_sb[:].rearrange("p q -> p 1 q").broadcast_to([128, 2, 256]))
                        nc.scalar.activation(x2[:], z2b[:], AF.Relu)
                    else:
                        nc.scalar.activation(x2[:], z2[:], AF.Relu)

                    # ---- L3 ----
                    agg3 = psB.tile([128, 512], f32, tag="B")
                    for g in (0, 1):
                        for k in (0, 1):
                            nc.tensor.matmul(
                                agg3[:, g * 256:(g + 1) * 256],
                                x2[:, k * 256 + g * 128: k * 256 + (g + 1) * 128],
                                AT_sb[:, k * 256:(k + 1) * 256],
                                start=(k == 0), stop=(k == 1))
                    agg3_sb = wk.tile([128, 512], bf16, tag="agg3")
                    nc.vector.tensor_copy(agg3_sb[:], agg3[:])
                    z3 = psA.tile([128, 1024], f32, tag="A")
                    for g in (0, 1):
                        for m in (0, 1):
                            nc.tensor.matmul(
                                z3[:, g * 512 + m * 256: g * 512 + (m + 1) * 256],
                                agg3_sb[:, g * 256 + m * 128: g * 256 + (m + 1) * 128],
                                W3_sb[:], start=True, stop=True)
                    x3 = wk.tile([128, 1024], bf16, tag="x3")
                    if gcn_bias:
                        z3b = wk.tile([128, 1024], f32, tag="z3b")
                        nc.vector.tensor_add(
                            z3b[:].rearrange("p (gm q) -> p gm q", gm=4),
                            z3[:].rearrange("p (gm q) -> p gm q", gm=4),
                            bb3_sb[:].rearrange("p q -> p 1 q").broadcast_to([128, 4, 256]))
                        nc.scalar.activation(x3[:], z3b[:], AF.Relu)
                    else:
                        nc.scalar.activation(x3[:], z3[:], AF.Relu)

                    # ---- L4 ----
                    agg4 = psA.tile([128, 1024], f32, tag="A")
                    for g in (0, 1):
                        for mc in (0, 1):
                            for k in (0, 1):
                                nc.tensor.matmul(
                                    agg4[:, g * 512 + mc * 256: g * 512 + (mc + 1) * 256],
                                    x3[:, g * 512 + k * 256 + mc * 128:
                                          g * 512 + k * 256 + (mc + 1) * 128],
                                    AT_sb[:, k * 256:(k + 1) * 256],
                                    start=(k == 0), stop=(k == 1))
                    agg4_sb = wk.tile([128, 1024], bf16, tag="agg4")
                    nc.vector.tensor_copy(agg4_sb[:], agg4[:])
                    z4 = psA.tile([128, 1024], f32, tag="A")
                    for g in (0, 1):
                        for mo in (0, 1):
                            for k in (0, 1):
                                nc.tensor.matmul(
                                    z4[:, g * 512 + mo * 256: g * 512 + (mo + 1) * 256],
                                    W4_sb[:, k * 256 + mo * 128: k * 256 + (mo + 1) * 128],
                                    agg4_sb[:, g * 512 + k * 256: g * 512 + (k + 1) * 256],
                                    start=(k == 0), stop=(k == 1))
                    # relu (scalar) + sum-pool over nodes via vector reduce;
                    # 1/N folded into WihT
                    x4 = wk.tile([128, 1024], bf16, tag="x4")
                    if gcn_bias:
                        for g in (0, 1):
                            for mo in (0, 1):
                                sl = slice(g * 512 + mo * 256, g * 512 + (mo + 1) * 256)
                                nc.scalar.activation(x4[:, sl], z4[:, sl], AF.Relu,
                                                     bias=b4c_sb[:, mo:mo + 1])
                    else:
                        nc.scalar.activation(x4[:], z4[:], AF.Relu)
                    with nc.allow_low_precision(reason="f32r pool accum (32-bit)"):
                        for g in (0, 1):
                            t_idx = 2 * j + g
                            for mo in (0, 1):
                                sl = slice(g * 512 + mo * 256, g * 512 + (mo + 1) * 256)
                                nc.vector.tensor_reduce(
                                    out=pooledT_sb[:, mo * 32 + t_idx:
                                                   mo * 32 + t_idx + 1],
                                    in_=x4[:, sl],
                                    axis=mybir.AxisListType.X, op=ALU.add)

            # ======= stage C: LSTM + fc + attention + head =======
            # gates laid out (i, o, g) x (dir) x h; i,o have 0.5 folded into
            # WihT so sigmoid(x) = 0.5*(1+tanh(x/2)) needs only tanh.
            with (
                tc.tile_pool(name="tail", bufs=1) as tl,
                tc.tile_pool(name="tailps_g", bufs=1, space="PSUM") as tpg,
                tc.tile_pool(name="tailps", bufs=2, space="PSUM") as tp,
            ):
                g_ps = tpg.tile([32, 1536], f32, tag="gates")
                for s in range(3):
                    for k in (0, 1):
                        nc.tensor.matmul(
                            g_ps[:, s * 512:(s + 1) * 512],
                            rf(pooledT_sb[:, k * 32:(k + 1) * 32]),
                            WihT_sb[:, k * 1536 + s * 512: k * 1536 + (s + 1) * 512],
                            start=(k == 0),
                            stop=(k == 1 and not lstm_bias))
                    if lstm_bias:
                        nc.tensor.matmul(g_ps[:, s * 512:(s + 1) * 512],
                                         ones_r[:],
                                         bihT_sb[:, s * 512:(s + 1) * 512],
                                         start=False, stop=True)
                th = tl.tile([32, 1536], f32)
                nc.scalar.activation(th[:], g_ps[:], AF.Tanh)
                # c2 = 2c = (1+tanh(i/2))*tanh(g) ; tc = tanh(c2 * 0.5)
                c2 = tl.tile([32, 512], f32)
                nc.vector.scalar_tensor_tensor(
                    out=c2[:], in0=th[:, 0:512], scalar=1.0,
                    in1=th[:, 1024:1536], op0=ALU.add, op1=ALU.mult)
                tc_sb = tl.tile([32, 512], f32)
                nc.scalar.activation(tc_sb[:], c2[:], AF.Tanh, scale=0.5)
                # h2 = 2h = (1+tanh(o/2))*tanh(c); the remaining 1/2 is folded
                # into fcW (pre-scaled 0.5 on host)
                h2 = tl.tile([32, 512], f32)
                nc.vector.scalar_tensor_tensor(
                    out=h2[:], in0=th[:, 512:1024], scalar=1.0,
                    in1=tc_sb[:], op0=ALU.add, op1=ALU.mult)

                # transpose h2 -> hT [128, (k,t)]
                hT_ps = tp.tile([128, 128], f32, tag="small")
                for k in range(4):
                    nc.tensor.transpose(hT_ps[:, k * 32:(k + 1) * 32],
                                        h2[:, k * 128:(k + 1) * 128],
                                        ident[0:32, 0:32])
                hT_sb = tl.tile([128, 128], f32r)
                nc.vector.tensor_copy(hT_sb[:], hT_ps[:])

                # embT [128, (mo,t)] = fcW^T @ hT
                embT_ps = tp.tile([128, 64], f32, tag="small")
                for mo in (0, 1):
                    for k in range(4):
                        nc.tensor.matmul(
                            embT_ps[:, mo * 32:(mo + 1) * 32],
                            fcW_sb[:, k * 256 + mo * 128: k * 256 + (mo + 1) * 128],
                            hT_sb[:, k * 32:(k + 1) * 32],
                            start=(k == 0), stop=(k == 3))
                embT_sb = tl.tile([128, 64], f32r)
                if fc_bias:
                    for mo in (0, 1):
                        nc.scalar.activation(embT_sb[:, mo * 32:(mo + 1) * 32],
                                             embT_ps[:, mo * 32:(mo + 1) * 32],
                                             AF.Identity,
                                             bias=fcb_sb[:, mo:mo + 1])
                else:
                    nc.vector.tensor_copy(embT_sb[:], embT_ps[:])

                # attention scores [1, 32]; attn_b dropped (softmax shift-inv);
                # no max-subtract (scores are O(1) by construction)
                sc_ps = tp.tile([1, 32], f32, tag="small")
                for mo in (0, 1):
                    nc.tensor.matmul(sc_ps[:], attnW_sb[:, mo:mo + 1],
                                     embT_sb[:, mo * 32:(mo + 1) * 32],
                                     start=(mo == 0), stop=(mo == 1))
                ex = tl.tile([1, 32], f32)
                ssum = tl.tile([1, 1], f32)
                nc.scalar.activation(ex[:], sc_ps[:], AF.Exp, accum_out=ssum[:])
                rs = tl.tile([1, 1], f32)
                nc.vector.reciprocal(rs[:], ssum[:])
                w_row = tl.tile([1, 32], f32)
                nc.vector.tensor_scalar_mul(w_row[:], ex[:], rs[:])
                w_bc = tp.tile([128, 32], f32, tag="wbc")
                nc.tensor.matmul(w_bc[:], ones_row[:], w_row[:],
                                 start=True, stop=True)

                # x_weighted[m] = sum_t embT[m,t] * w[t]  (fused mul+accum)
                xw_scr = tl.tile([128, 64], f32r)
                xw_col = tl.tile([128, 2], f32r)
                with nc.allow_low_precision(reason="f32r weighted-sum accum"):
                    for mo in (0, 1):
                        nc.vector.scalar_tensor_tensor(
                            out=xw_scr[:, mo * 32:(mo + 1) * 32],
                            in0=embT_sb[:, mo * 32:(mo + 1) * 32], scalar=1.0,
                            in1=w_bc[:], op0=ALU.mult, op1=ALU.mult,
                            accum_out=xw_col[:, mo:mo + 1])

                # head: out = xw @ out_W (+ out_b)
                fin_ps = tp.tile([1, 512], f32, tag="small")
                for mo in (0, 1):
                    nc.tensor.matmul(fin_ps[:], xw_col[:, mo:mo + 1],
                                     outW_sb[:, mo * 512:(mo + 1) * 512],
                                     start=(mo == 0),
                                     stop=(mo == 1 and not out_bias))
                if out_bias:
                    nc.tensor.matmul(fin_ps[:], ones_r[0:1, 0:1], outb_sb[:],
                                     start=False, stop=True)
                fin_sb = tl.tile([1, 512], f32)
                nc.vector.tensor_copy(fin_sb[:], fin_ps[:])
                nc.sync.dma_start(out=out_d.ap(), in_=fin_sb[:])

    nc.compile()
    return nc


def _get_nc(flags):
    key = tuple(sorted(flags.items()))
    if key not in _CACHE:
        _CACHE[key] = _build(flags)
    return _CACHE[key]


def kernel(**inputs):
    from concourse import bass_utils

    bf = ml_dtypes.bfloat16
    inp = {k: np.asarray(v) for k, v in inputs.items()}
    flags = {
        "gcn_bias": any(np.any(inp[f"b{i}"]) for i in (1, 2, 3, 4)),
        "lstm_bias": any(np.any(inp[k]) for k in
                         ("b_ih_f", "b_hh_f", "b_ih_b", "b_hh_b")),
        "fc_bias": bool(np.any(inp["fc_b"])),
        "out_bias": bool(np.any(inp["out_b"])),
    }
    nc = _get_nc(flags)

    f32 = np.float32
    W1 = inp["W1"].astype(f32)
    W2 = inp["W2"].astype(f32)
    W1blk = np.zeros((128, 128), f32)
    W1blk[:64, :64] = W1
    W1blk[64:, 64:] = W1
    W2blk = np.zeros((128, 256), f32)
    W2blk[:64, :128] = W2
    W2blk[64:, 128:] = W2
    W4p = inp["W4"].astype(f32).reshape(2, 128, 256).transpose(1, 0, 2).reshape(128, 512)

    # WihT: [co, g'*512 + d*256 + h], gate order (i, o, g); i,o scaled 0.5
    # (sigmoid-from-tanh), everything scaled 1/N (mean-pool folded in)
    M = np.zeros((256, 1536), f32)
    for di, dname in enumerate(("f", "b")):
        Wih = inp[f"W_ih_{dname}"].astype(f32)  # [4H, H] rows gate*256+h
        for gdst, (gsrc, sc) in enumerate([(0, 0.5), (3, 0.5), (2, 1.0)]):
            M[:, gdst * 512 + di * 256: gdst * 512 + (di + 1) * 256] = \
                Wih[gsrc * 256:(gsrc + 1) * 256, :].T * (sc / N)
    WihTp = M.reshape(2, 128, 1536).transpose(1, 0, 2).reshape(128, 3072)

    fcWp = (inp["fc_W"].astype(f32) * 0.5).reshape(4, 128, 256) \
        .transpose(1, 0, 2).reshape(128, 1024)
    attnWp = np.ascontiguousarray(inp["attn_W"].astype(f32).reshape(2, 128).T)
    outWp = inp["out_W"].astype(f32).reshape(2, 128, 512) \
        .transpose(1, 0, 2).reshape(128, 1024)

    base = {
        "edge_index": np.ascontiguousarray(inp["edge_index"].astype(np.int32)),
        "W1blk": np.ascontiguousarray(W1blk.astype(bf)),
        "W2blk": np.ascontiguousarray(W2blk.astype(bf)),
        "W3p": np.ascontiguousarray(inp["W3"].astype(f32).astype(bf)),
        "W4p": np.ascontiguousarray(W4p.astype(bf)),
        "WihTp": np.ascontiguousarray(WihTp),
        "fcWp": np.ascontiguousarray(fcWp),
        "attnWp": attnWp,
        "outWp": np.ascontiguousarray(outWp),
    }
    if flags["gcn_bias"]:
        b1 = inp["b1"].astype(f32)
        b2 = inp["b2"].astype(f32)
        b3 = inp["b3"].astype(f32)
        b4 = inp["b4"].astype(f32)
        base["bb1"] = np.ascontiguousarray(
            np.tile(np.concatenate([b1, b1]), (128, 1)))
        base["bb2"] = np.ascontiguousarray(
            np.tile(np.concatenate([b2, b2]), (128, 1)))
        base["bb3"] = np.ascontiguousarray(np.tile(b3, (128, 1)))
        base["b4col"] = np.ascontiguousarray(b4.reshape(2, 128).T)
    if flags["lstm_bias"]:
        bihT = np.zeros((1, 1536), f32)
        for di, dname in enumerate(("f", "b")):
            bsum = (inp[f"b_ih_{dname}"] + inp[f"b_hh_{dname}"]).astype(f32)
            for gdst, (gsrc, sc) in enumerate([(0, 0.5), (3, 0.5), (2, 1.0)]):
                bihT[0, gdst * 512 + di * 256: gdst * 512 + (di + 1) * 256] = \
                    bsum[gsrc * 256:(gsrc + 1) * 256] * sc
        base["bihT"] = bihT
    if flags["fc_bias"]:
        base["fcb_col"] = np.ascontiguousarray(
            inp["fc_b"].astype(f32).reshape(2, 128).T)
    if flags["out_bias"]:
        base["outb_row"] = np.ascontiguousarray(
            inp["out_b"].astype(f32).reshape(1, 512))

    # x0: [p, j*256 + k*128 + g*64 + c] = data[2j+g, k*128+p, c], bf16
    data = inp["data"].astype(f32)
    in_maps = []
    for c in range(NCORES):
        v = data[c].reshape(NPAIR, 2, 2, 128, F)          # [j, g, k, p, c]
        x0 = v.transpose(3, 0, 2, 1, 4).reshape(128, NPAIR * 256)
        in_maps.append(dict(base, x0=np.ascontiguousarray(x0.astype(bf))))

    global LAST_RESULT
    res = bass_utils.run_bass_kernel_spmd(nc, in_maps,
                                          core_ids=list(range(NCORES)),
                                          **RUN_KWARGS)
    LAST_RESULT = res
    return np.concatenate([r["out"] for r in res.results], axis=0)


if __name__ == "__main__":
    import reference
    inputs = {k: np.asarray(v) for k, v in reference.setup_inputs().items()}
    got = kernel(**inputs)
    print(got.shape, got.dtype)
